# revision 11
# baseline (speedup 1.0000x reference)
"""CapsNet routing layer (nn_CapsLayer) on 8 Trainium2 NeuronCores — v14.

reference:
    u_hat = einsum("ncoi,bci->bnco", W[0], x)         # B,N,C,O = 1024,2,512,64
    3 dynamic-routing iterations (softmax over n, weighted sum over c,
    squash, agreement update); returns v from iteration 3.

v14 restructure (from v13's trace: every AR-gated ACT op convoys with
the GEMM's PSUM-evacuation copies on the ACT FIFO; PE+DVE+ACT all
serialized behind AllReduce latency):
  - squash factor folded into the d-update as a per-partition scalar:
    the y-pass multiplies u by the RAW (scaled) AR result, and the
    squash factor f = sqrt(sq)/(1+sq) is applied after the o-reduce
    via scalar_tensor_tensor. The factor is computed on DVE with a
    bitwise-magic rsqrt + 2 Newton iterations (4.7e-6 max rel err),
    so NO ACT op sits on the AR critical path; ACT keeps only the
    sigmoid materialization and the pg->u copies (one table set, no
    Sigmoid<->Sqrt swaps).
  - pairs 0,1 compute their U-partials with DVE copy+tree in the
    otherwise-idle head; pairs 2,3 use the PE s_acc stream.
  - step_A split into A_y / A_s so mid-routing work (U-trees, GEMMs)
    interleaves at the right DVE FIFO positions.
  - emission order hand-matched to readiness order on every FIFO
    (PE, DVE, ACT, gpsimd/CC); collectives alone on the gpsimd queue.
  - all DMAs on the sync queue; W interleaved with chunk 0's x groups.
"""
import os
import sys
import types

sys.path.insert(0, "/opt/trn_rl_repo")

import numpy as np
import concourse.bass as bass
import concourse.mybir as mybir
import concourse.tile as tile
from concourse.bass_utils import run_bass_kernel_spmd

B, NCAPS, C, ICH, OCH = 1024, 2, 512, 256, 64
ITERATIONS = 3
NCORES = 8
CPC = C // NCORES            # in-caps per core = 64
NBCH = 8                     # batch chunks
BCH = B // NBCH              # samples per chunk = 128
KH = 2                       # K halves (ICH = 2*128)
CG = 8                       # c's per GEMM/DMA group
NO = NCAPS * OCH             # 128
NSL = 4                      # chunk v-slots
SC = 512.0                   # vts scale keeping fp16 products small

FP32 = mybir.dt.float32
FP16 = mybir.dt.float16
I32 = mybir.dt.int32
ADD = mybir.AluOpType.add
MULT = mybir.AluOpType.mult
SUB = mybir.AluOpType.subtract
SHR = mybir.AluOpType.logical_shift_right
AF = mybir.ActivationFunctionType

LAST_EXEC_NS = None


def _install_profile_hook():
    """antenv.axon_hooks is absent in this image; recreate it so
    run_bass_kernel_spmd(trace=True)/BASS_TRACE can report exec_time_ns."""
    if "antenv.axon_hooks" in sys.modules:
        return
    mod = types.ModuleType("antenv.axon_hooks")
    mod._hook = None
    mod.set_axon_ntff_profile_hook = lambda h: setattr(mod, "_hook", h)
    mod.get_axon_ntff_profile_hook = lambda: mod._hook
    sys.modules["antenv.axon_hooks"] = mod
    try:
        from trn_agent_boot.trn_boot import _ntff_profile_via_ctypes

        hook = _ntff_profile_via_ctypes("/opt/axon/libaxon_pjrt.so")
        if hook is not None:
            mod._hook = hook
    except Exception:
        pass


def _split_sync_waits(nc, max_waits=1):
    """walrus setupSyncWait rejects instructions with more than one sem
    wait; hoist extras onto same-engine InstNoOp's placed just before."""
    for f in nc.m.functions:
        for bb in f.blocks:
            out = []
            changed = False
            for inst in bb.instructions:
                si = inst.sync_info
                waits = list(si.on_wait) if si is not None and si.on_wait else []
                if len(waits) > max_waits:
                    extra, keep = waits[:-max_waits], waits[-max_waits:]
                    for g, w in enumerate(extra):
                        out.append(
                            mybir.InstNoOp(
                                name=f"{inst.name}_wsplit{g}",
                                engine=inst.engine,
                                bass_nofuse=True,
                                sync_info=mybir.SyncInfo(on_wait=[w], on_update=[]),
                            )
                        )
                    inst.sync_info = mybir.SyncInfo(
                        on_wait=keep,
                        on_update=list(si.on_update) if si.on_update else [],
                    )
                    changed = True
                out.append(inst)
            if changed:
                bb.instructions = out
    return nc


def build_kernel(split_waits=True):
    nc = bass.Bass(
        "TRN2", target_bir_lowering=False, debug=False, num_devices=NCORES
    )
    xt = nc.dram_tensor("xt", [KH, 128, NBCH, CPC, BCH], FP16, kind="ExternalInput").ap()
    wt = nc.dram_tensor("wt", [KH, 128, CPC, NO], FP16, kind="ExternalInput").ap()
    out = nc.dram_tensor("out", [B, NCAPS, OCH], FP32, kind="ExternalOutput").ap()

    with tile.TileContext(nc) as tc:
        with (
            tc.tile_pool(name="xin", bufs=2) as xpool,
            tc.tile_pool(name="psum", bufs=3, space="PSUM") as pspool,
            tc.tile_pool(name="sacc", bufs=2, space="PSUM") as sapool,
            tc.tile_pool(name="ubuf", bufs=7) as upool,
            tc.tile_pool(name="dram", bufs=8, space="DRAM") as drpool,
        ):
            # resident W; its per-group DMAs are emitted inside chunk 0's
            # group loop so W and x arrive in consumption order
            wsb = []
            for h in range(KH):
                t = nc.alloc_sbuf_tensor(f"w{h}", [128, CPC * NO], FP16).ap()
                wsb.append(t)

            # ---- CC warmup: a tiny AllReduce kicks off the ~50us
            # collective-infrastructure init under the GEMM phase.
            wu = nc.alloc_sbuf_tensor("wu", [128, 1], FP32).ap()
            nc.vector.memset(wu[:], 0.0)
            wi = drpool.tile([128, 1], FP32, tag="wi")
            wo = drpool.tile([128, 1], FP32, tag="wo", addr_space="Shared")
            nc.sync.dma_start(wi[:], wu[:])
            nc.gpsimd.collective_compute(
                "AllReduce",
                ADD,
                replica_groups=[list(range(NCORES))],
                ins=[wi[:].opt()],
                outs=[wo[:].opt()],
            )

            # persistent routing state / scratch
            d_all = nc.alloc_sbuf_tensor("d_all", [128, NBCH, CPC], FP16).ap()
            wsc = nc.alloc_sbuf_tensor("wsc0", [128, NCAPS, CPC, OCH], FP16).ap()
            P = 2
            sgm = [nc.alloc_sbuf_tensor(f"sgm{p}", [128, CPC, OCH], FP16).ap() for p in range(P)]
            dds = [nc.alloc_sbuf_tensor(f"dds{p}", [128, CPC], FP16).ap() for p in range(P)]
            tmn = [nc.alloc_sbuf_tensor(f"tmn{p}", [128, CPC], FP16).ap() for p in range(P)]
            vts = [nc.alloc_sbuf_tensor(f"vt{s}", [128, NCAPS, OCH], FP16).ap() for s in range(NSL)]
            # pair-level tensors, one slot per pair, z = chunk-in-pair
            QP = 4
            Upr = [nc.alloc_sbuf_tensor(f"Upr{q}", [128, 2, NCAPS, OCH], FP16).ap() for q in range(QP)]
            ssp = [nc.alloc_sbuf_tensor(f"ssp{q}", [128, 2, NCAPS, OCH], FP16).ap() for q in range(QP)]
            pfp = [nc.alloc_sbuf_tensor(f"pfp{q}", [128, 2, NCAPS, OCH], FP16).ap() for q in range(QP)]
            s2p = [nc.alloc_sbuf_tensor(f"s2p{q}", [128, 2, NCAPS, OCH], FP32).ap() for q in range(QP)]
            sab = [nc.alloc_sbuf_tensor(f"sab{q}", [128, 2, NO], FP16).ap() for q in range(QP)]
            sqp = [nc.alloc_sbuf_tensor(f"sqp{q}", [128, 4], FP32).ap() for q in range(QP)]
            rsq = [nc.alloc_sbuf_tensor(f"rsq{q}", [128, 4], FP32).ap() for q in range(QP)]
            nt1 = [nc.alloc_sbuf_tensor(f"nt1{q}", [128, 4], FP32).ap() for q in range(QP)]
            dnp = [nc.alloc_sbuf_tensor(f"dnp{q}", [128, 4], FP32).ap() for q in range(QP)]
            rcp = [nc.alloc_sbuf_tensor(f"rcp{q}", [128, 4], FP32).ap() for q in range(QP)]
            gsq = [nc.alloc_sbuf_tensor(f"gsq{q}", [128, 4], FP32).ap() for q in range(QP)]
            fsg = [nc.alloc_sbuf_tensor(f"fsg{q}", [128, 4], FP32).ap() for q in range(QP)]
            vfp = [nc.alloc_sbuf_tensor(f"vfp{q}", [128, 2, NCAPS, OCH], FP32).ap() for q in range(QP)]

            # consts: rsqrt bithack + signed factor scales per (z, n)
            onei = nc.alloc_sbuf_tensor("onei", [128, 4], I32).ap()
            magi = nc.alloc_sbuf_tensor("magi", [128, 4], I32).ap()
            nc.vector.memset(onei[:], 1)
            nc.vector.memset(magi[:], 0x5F3759DF)
            sgnS0 = nc.alloc_sbuf_tensor("sgnS0", [128, 4], FP32).ap()
            sgnS1 = nc.alloc_sbuf_tensor("sgnS1", [128, 4], FP32).ap()
            for z in range(2):
                # index 2z+n; + for n=0, - for n=1
                nc.vector.memset(sgnS0[:, 2 * z : 2 * z + 1], 0.25 * SC)
                nc.vector.memset(sgnS0[:, 2 * z + 1 : 2 * z + 2], -0.25 * SC)
                nc.vector.memset(sgnS1[:, 2 * z : 2 * z + 1], SC)
                nc.vector.memset(sgnS1[:, 2 * z + 1 : 2 * z + 2], -SC)

            def allreduce_pair(src, dst):
                bi = drpool.tile([128, 2 * NO], FP16, tag="bi")
                bo = drpool.tile([128, 2 * NO], FP16, tag="bo", addr_space="Shared")
                nc.sync.dma_start(bi[:], src)
                nc.gpsimd.collective_compute(
                    "AllReduce",
                    ADD,
                    replica_groups=[list(range(NCORES))],
                    ins=[bi[:].opt()],
                    outs=[bo[:].opt()],
                )
                nc.sync.dma_start(dst, bo[:])

            def tree(t, axis_len, o_axis):
                """in-place binary-tree sum over c (o_axis=False) or o."""
                lv = axis_len // 2
                while lv >= 1:
                    if o_axis:
                        a = t[:, :, :, 0:lv]
                        b = t[:, :, :, lv : 2 * lv]
                    else:
                        a = t[:, :, 0:lv, :]
                        b = t[:, :, lv : 2 * lv, :]
                    nc.vector.tensor_tensor(a, a, b, op=ADD)
                    lv //= 2

            def gemm_chunk(bk, us, with_sacc):
                u = upool.tile([128, NCAPS, CPC, OCH], FP16, tag="u")
                us[bk] = u
                if with_sacc:
                    sacc = sapool.tile([128, NO], FP32, tag="sacc", name="sacc")
                else:
                    sacc = None
                for cg in range(CPC // CG):
                    c0 = cg * CG
                    xtt = []
                    for h in range(KH):
                        if bk == 0:
                            nc.sync.dma_start(
                                wsb[h][:, c0 * NO : (c0 + CG) * NO],
                                wt[h, :, c0 : c0 + CG, :].rearrange("i c f -> i (c f)"),
                            )
                        t = xpool.tile([128, CG, BCH], FP16, tag=f"x{h}")
                        nc.sync.dma_start(t[:], xt[h, :, bk, c0 : c0 + CG, :])
                        xtt.append(t)
                    pg = pspool.tile([BCH, CG, NO], FP32, tag="pg")
                    for j in range(CG):
                        c = c0 + j
                        for h in range(KH):
                            lhs = xtt[h][:, j, :]
                            rhs = wsb[h][:, c * NO : (c + 1) * NO]
                            nc.tensor.matmul(
                                pg[:, j, :], lhsT=lhs, rhs=rhs,
                                start=(h == 0), stop=(h == KH - 1),
                            )
                            if with_sacc:
                                nc.tensor.matmul(
                                    sacc[:], lhsT=lhs, rhs=rhs,
                                    start=(c == 0 and h == 0),
                                    stop=(c == CPC - 1 and h == KH - 1),
                                    skip_group_check=True,
                                )
                    nc.scalar.copy(
                        u[:, :, c0 : c0 + CG, :],
                        pg[:].rearrange("b c (n o) -> b n c o", n=NCAPS),
                    )
                return u, sacc

            def utree_dve(q, z, us):
                """chunk U-partial on DVE (copy + c-tree) into sab."""
                qp = q % QP
                nc.vector.tensor_copy(wsc[:], us[2 * q + z][:])
                tree(wsc, CPC, False)
                nc.vector.tensor_copy(
                    sab[qp][:, z, :].rearrange("p (n o) -> p n o", n=NCAPS),
                    wsc[:, :, 0, :],
                )

            def sacc_out(q, z, sacc):
                nc.scalar.copy(sab[q % QP][:, z, :], sacc[:])

            def ar0(q):
                qp = q % QP
                allreduce_pair(
                    sab[qp][:].rearrange("p z f -> p (z f)"),
                    Upr[qp][:].rearrange("p z n o -> p (z n o)"),
                )

            def factor_prep(q, s_in, qscale, sgnS):
                """vts = s_in/SC (per z); fsg = +-scale*factor on DVE only.
                factor = sqrt(q)/(1+q), q = qscale*sqp, via bithack rsqrt."""
                qp = q % QP
                for z in range(2):
                    bk = 2 * q + z
                    nc.vector.tensor_scalar(
                        vts[bk % NSL][:], s_in[:, z], 1.0 / SC, None, op0=MULT
                    )
                # sqp[:, 2z+n] = sum_o s^2 : square then 6-level tree
                s2 = s2p[qp][:].rearrange("p z n o -> p (z n) o")
                si = s_in.rearrange("p z n o -> p (z n) o")
                nc.vector.tensor_tensor(s2, si, si, op=MULT)
                lv = OCH // 2
                while lv >= 1:
                    nc.vector.tensor_tensor(
                        s2[:, :, 0:lv], s2[:, :, 0:lv], s2[:, :, lv : 2 * lv], op=ADD
                    )
                    lv //= 2
                nc.vector.tensor_copy(
                    sqp[qp][:].rearrange("p (zn one) -> p zn one", one=1),
                    s2[:, :, 0:1],
                )
                # r ~= rsqrt(sqp): magic - (bits >> 1), 2 Newton steps
                rb, qb = rsq[qp][:].bitcast(I32), sqp[qp][:].bitcast(I32)
                nc.vector.tensor_tensor(rb, qb, onei[:], op=SHR)
                nc.vector.tensor_tensor(rb, magi[:], rb, op=SUB)
                for _ in range(2):
                    nc.vector.tensor_tensor(nt1[qp][:], rsq[qp][:], rsq[qp][:], op=MULT)
                    nc.vector.tensor_tensor(nt1[qp][:], nt1[qp][:], sqp[qp][:], op=MULT)
                    nc.vector.tensor_scalar(
                        nt1[qp][:], nt1[qp][:], -0.5, 1.5, op0=MULT, op1=ADD
                    )
                    nc.vector.tensor_tensor(rsq[qp][:], rsq[qp][:], nt1[qp][:], op=MULT)
                # g = sqrt(sqp); dn = 1 + qscale*sqp; f = g/dn; fsg = f*sgnS
                nc.vector.tensor_tensor(gsq[qp][:], sqp[qp][:], rsq[qp][:], op=MULT)
                nc.vector.tensor_scalar(dnp[qp][:], sqp[qp][:], float(qscale), 1.0, op0=MULT, op1=ADD)
                nc.vector.reciprocal(rcp[qp][:], dnp[qp][:])
                nc.vector.tensor_tensor(gsq[qp][:], gsq[qp][:], rcp[qp][:], op=MULT)
                if sgnS is None:
                    return  # final: gsq = raw factor
                nc.vector.tensor_tensor(fsg[qp][:], gsq[qp][:], sgnS[:], op=MULT)

            def step_S0(q):
                """iteration-0: vts = U/SC; f0 = +-0.25*SC*2*factor0."""
                qp = q % QP
                factor_prep(q, Upr[qp][:], 0.25, sgnS0)

            def step_A_y(q, it):
                """y-pass: t = u*vts_bcast, o-tree, d += f0*t0 + f1*t1;
                then sigma on ACT."""
                qp = q % QP
                for z in range(2):
                    bk = 2 * q + z
                    u = us[bk]
                    vb = vts[bk % NSL][:].unsqueeze(2).broadcast_to((128, NCAPS, CPC, OCH))
                    nc.vector.tensor_tensor(wsc[:], u[:], vb, op=MULT)
                    tree(wsc, OCH, True)
                    t0 = wsc[:, 0, :, 0:1]
                    t1 = wsc[:, 1, :, 0:1]
                    nc.vector.tensor_scalar(
                        tmn[z][:].unsqueeze(2), t1, fsg[qp][:, 2 * z + 1 : 2 * z + 2],
                        None, op0=MULT,
                    )
                    if it == 1:
                        nc.vector.scalar_tensor_tensor(
                            d_all[:, bk, :].unsqueeze(2), t0,
                            fsg[qp][:, 2 * z : 2 * z + 1],
                            tmn[z][:].unsqueeze(2), op0=MULT, op1=ADD,
                        )
                    else:
                        nc.vector.scalar_tensor_tensor(
                            dds[z][:].unsqueeze(2), t0,
                            fsg[qp][:, 2 * z : 2 * z + 1],
                            tmn[z][:].unsqueeze(2), op0=MULT, op1=ADD,
                        )
                        nc.vector.tensor_tensor(
                            d_all[:, bk, :], d_all[:, bk, :], dds[z][:], op=ADD
                        )
                for z in range(2):
                    bk = 2 * q + z
                    db = d_all[:, bk, :].unsqueeze(2).broadcast_to((128, CPC, OCH))
                    nc.scalar.activation(sgm[z][:], db, AF.Sigmoid)

            def step_A_s(q, it):
                """s-pass: sigma-weighted sums over c, partial extract, AR."""
                qp = q % QP
                for z in range(2):
                    sb = sgm[z][:].unsqueeze(1).broadcast_to((128, NCAPS, CPC, OCH))
                    nc.vector.tensor_tensor(wsc[:], us[2 * q + z][:], sb, op=MULT)
                    tree(wsc, CPC, False)
                    nc.scalar.copy(pfp[qp][:, z], wsc[:, :, 0, :])
                allreduce_pair(
                    pfp[qp][:].rearrange("p z n o -> p (z n o)"),
                    ssp[qp][:].rearrange("p z n o -> p (z n o)"),
                )

            def step_B(q, it):
                """post-AR: sigmoid-complement fix; factor prep (or final)."""
                qp = q % QP
                nc.vector.tensor_tensor(
                    ssp[qp][:, :, 1, :], Upr[qp][:, :, 1, :], ssp[qp][:, :, 1, :],
                    op=SUB,
                )
                if it < ITERATIONS - 1:
                    factor_prep(q, ssp[qp][:], 1.0, sgnS1)
                else:
                    factor_prep(q, ssp[qp][:], 1.0, None)
                    mb = (
                        gsq[qp][:]
                        .rearrange("p (z n) -> p z n", z=2)
                        .unsqueeze(3)
                        .broadcast_to((128, 2, NCAPS, OCH))
                    )
                    nc.vector.tensor_tensor(vfp[qp][:], ssp[qp][:], mb, op=MULT)
                    for z in range(2):
                        bk = 2 * q + z
                        nc.sync.dma_start(
                            out[bk * BCH : (bk + 1) * BCH, :, :], vfp[qp][:, z]
                        )

            # ---- schedule: emission matched to per-FIFO readiness order
            us = {}
            gemm_chunk(0, us, False); utree_dve(0, 0, us)
            gemm_chunk(1, us, False); utree_dve(0, 1, us); ar0(0)
            gemm_chunk(2, us, False)
            gemm_chunk(3, us, False)
            step_S0(0)
            step_A_y(0, 1)
            utree_dve(1, 0, us); utree_dve(1, 1, us); ar0(1)
            step_A_s(0, 1)
            _, sc4 = gemm_chunk(4, us, True); sacc_out(2, 0, sc4)
            _, sc5 = gemm_chunk(5, us, True); sacc_out(2, 1, sc5); ar0(2)
            step_S0(1); step_A_y(1, 1); step_A_s(1, 1)
            step_B(0, 1); step_A_y(0, 2)
            _, sc6 = gemm_chunk(6, us, True); sacc_out(3, 0, sc6)
            step_A_s(0, 2)
            _, sc7 = gemm_chunk(7, us, True); sacc_out(3, 1, sc7); ar0(3)
            step_B(1, 1); step_A_y(1, 2); step_A_s(1, 2)
            step_B(0, 2)
            step_S0(2); step_A_y(2, 1); step_A_s(2, 1)
            step_B(1, 2)
            step_S0(3); step_A_y(3, 1); step_A_s(3, 1)
            step_B(2, 1); step_A_y(2, 2); step_A_s(2, 2)
            step_B(3, 1); step_A_y(3, 2); step_A_s(3, 2)
            step_B(2, 2)
            step_B(3, 2)

    if split_waits:
        _split_sync_waits(nc)
    return nc


def _prep_inputs(x, W):
    x = np.ascontiguousarray(x, dtype=np.float32)
    W0 = np.ascontiguousarray(W.reshape(NCAPS, C, OCH, ICH), dtype=np.float32)
    xt_cores, wt_cores = [], []
    for k in range(NCORES):
        cs = k * CPC
        xc = x[:, cs : cs + CPC, :]  # (B, 64, 256)
        x6 = xc.reshape(NBCH, BCH, CPC, KH, 128)
        xtc = np.ascontiguousarray(x6.transpose(3, 4, 0, 2, 1)).astype(np.float16)
        xt_cores.append(xtc)
        Wc = W0[:, cs : cs + CPC]  # (2, 64, 64, 256) [n,c,o,i]
        w5 = Wc.reshape(NCAPS, CPC, OCH, KH, 128)
        wtc = np.ascontiguousarray(w5.transpose(3, 4, 1, 0, 2)).reshape(
            KH, 128, CPC, NO
        ).astype(np.float16)
        wt_cores.append(wtc)
    return xt_cores, wt_cores


_NC_CACHE = {}


def kernel(x, W):
    global LAST_EXEC_NS
    _install_profile_hook()
    if "nc" not in _NC_CACHE:
        _NC_CACHE["nc"] = build_kernel()
    nc = _NC_CACHE["nc"]
    xtc, wtc = _prep_inputs(np.asarray(x), np.asarray(W))
    in_maps = [{"xt": xtc[k], "wt": wtc[k]} for k in range(NCORES)]
    trace = bool(os.environ.get("CAPS_TRACE"))
    res = run_bass_kernel_spmd(nc, in_maps, list(range(NCORES)), trace=trace)
    LAST_EXEC_NS = res.exec_time_ns
    return res.results[0]["out"].astype(np.float32)


# revision 14
# speedup vs baseline: 1.0349x; 1.0349x over previous
"""CapsNet routing layer (nn_CapsLayer) on 8 Trainium2 NeuronCores — v14.

reference:
    u_hat = einsum("ncoi,bci->bnco", W[0], x)         # B,N,C,O = 1024,2,512,64
    3 dynamic-routing iterations (softmax over n, weighted sum over c,
    squash, agreement update); returns v from iteration 3.

v14 restructure (from v13's trace: every AR-gated ACT op convoys with
the GEMM's PSUM-evacuation copies on the ACT FIFO; PE+DVE+ACT all
serialized behind AllReduce latency):
  - squash factor folded into the d-update as a per-partition scalar:
    the y-pass multiplies u by the RAW (scaled) AR result, and the
    squash factor f = sqrt(sq)/(1+sq) is applied after the o-reduce
    via scalar_tensor_tensor. The factor is computed on DVE with a
    bitwise-magic rsqrt + 2 Newton iterations (4.7e-6 max rel err),
    so NO ACT op sits on the AR critical path; ACT keeps only the
    sigmoid materialization and the pg->u copies (one table set, no
    Sigmoid<->Sqrt swaps).
  - pairs 0,1 compute their U-partials with DVE copy+tree in the
    otherwise-idle head; pairs 2,3 use the PE s_acc stream.
  - step_A split into A_y / A_s so mid-routing work (U-trees, GEMMs)
    interleaves at the right DVE FIFO positions.
  - emission order hand-matched to readiness order on every FIFO
    (PE, DVE, ACT, gpsimd/CC); collectives alone on the gpsimd queue.
  - all DMAs on the sync queue; W interleaved with chunk 0's x groups.
"""
import os
import sys
import types

sys.path.insert(0, "/opt/trn_rl_repo")

import numpy as np
import concourse.bass as bass
import concourse.mybir as mybir
import concourse.tile as tile
from concourse.bass_utils import run_bass_kernel_spmd

B, NCAPS, C, ICH, OCH = 1024, 2, 512, 256, 64
ITERATIONS = 3
NCORES = 8
CPC = C // NCORES            # in-caps per core = 64
NBCH = 8                     # batch chunks
BCH = B // NBCH              # samples per chunk = 128
KH = 2                       # K halves (ICH = 2*128)
CG = 8                       # c's per GEMM/DMA group
NO = NCAPS * OCH             # 128
NSL = 4                      # chunk v-slots
SC = 512.0                   # vts scale keeping fp16 products small

FP32 = mybir.dt.float32
FP16 = mybir.dt.float16
I32 = mybir.dt.int32
ADD = mybir.AluOpType.add
MULT = mybir.AluOpType.mult
SUB = mybir.AluOpType.subtract
SHR = mybir.AluOpType.logical_shift_right
AF = mybir.ActivationFunctionType

LAST_EXEC_NS = None


def _install_profile_hook():
    """antenv.axon_hooks is absent in this image; recreate it so
    run_bass_kernel_spmd(trace=True)/BASS_TRACE can report exec_time_ns."""
    if "antenv.axon_hooks" in sys.modules:
        return
    mod = types.ModuleType("antenv.axon_hooks")
    mod._hook = None
    mod.set_axon_ntff_profile_hook = lambda h: setattr(mod, "_hook", h)
    mod.get_axon_ntff_profile_hook = lambda: mod._hook
    sys.modules["antenv.axon_hooks"] = mod
    try:
        from trn_agent_boot.trn_boot import _ntff_profile_via_ctypes

        hook = _ntff_profile_via_ctypes("/opt/axon/libaxon_pjrt.so")
        if hook is not None:
            mod._hook = hook
    except Exception:
        pass


def _split_sync_waits(nc, max_waits=1):
    """walrus setupSyncWait rejects instructions with more than one sem
    wait; hoist extras onto same-engine InstNoOp's placed just before."""
    for f in nc.m.functions:
        for bb in f.blocks:
            out = []
            changed = False
            for inst in bb.instructions:
                si = inst.sync_info
                waits = list(si.on_wait) if si is not None and si.on_wait else []
                if len(waits) > max_waits:
                    extra, keep = waits[:-max_waits], waits[-max_waits:]
                    for g, w in enumerate(extra):
                        out.append(
                            mybir.InstNoOp(
                                name=f"{inst.name}_wsplit{g}",
                                engine=inst.engine,
                                bass_nofuse=True,
                                sync_info=mybir.SyncInfo(on_wait=[w], on_update=[]),
                            )
                        )
                    inst.sync_info = mybir.SyncInfo(
                        on_wait=keep,
                        on_update=list(si.on_update) if si.on_update else [],
                    )
                    changed = True
                out.append(inst)
            if changed:
                bb.instructions = out
    return nc


def build_kernel(split_waits=True):
    nc = bass.Bass(
        "TRN2", target_bir_lowering=False, debug=False, num_devices=NCORES
    )
    xt = nc.dram_tensor("xt", [KH, 128, NBCH, CPC, BCH], FP16, kind="ExternalInput").ap()
    wt = nc.dram_tensor("wt", [KH, 128, CPC, NO], FP16, kind="ExternalInput").ap()
    out = nc.dram_tensor("out", [B, NCAPS, OCH], FP32, kind="ExternalOutput").ap()

    with tile.TileContext(nc) as tc:
        with (
            tc.tile_pool(name="xin", bufs=2) as xpool,
            tc.tile_pool(name="psum", bufs=3, space="PSUM") as pspool,
            tc.tile_pool(name="sacc", bufs=2, space="PSUM") as sapool,
            tc.tile_pool(name="ubuf", bufs=7) as upool,
            tc.tile_pool(name="dram", bufs=8, space="DRAM") as drpool,
        ):
            # resident W on the scalar DMA queue (ACT is otherwise idle
            # at t=0 and the first copies depend on W-fed matmuls anyway)
            wsb = []
            for h in range(KH):
                t = nc.alloc_sbuf_tensor(f"w{h}", [128, CPC * NO], FP16).ap()
                wsb.append(t)
            for cg in range(CPC // CG):
                c0 = cg * CG
                for h in range(KH):
                    nc.scalar.dma_start(
                        wsb[h][:, c0 * NO : (c0 + CG) * NO],
                        wt[h, :, c0 : c0 + CG, :].rearrange("i c f -> i (c f)"),
                    )

            # ---- CC warmup: a tiny AllReduce at t~0 kicks off the ~55us
            # collective-infrastructure init under the GEMM phase.
            wu = nc.alloc_sbuf_tensor("wu", [128, 1], FP32).ap()
            nc.vector.memset(wu[:], 0.0)
            wi = drpool.tile([128, 1], FP32, tag="wi")
            wo = drpool.tile([128, 1], FP32, tag="wo", addr_space="Shared")
            nc.sync.dma_start(wi[:], wu[:])
            nc.gpsimd.collective_compute(
                "AllReduce",
                ADD,
                replica_groups=[list(range(NCORES))],
                ins=[wi[:].opt()],
                outs=[wo[:].opt()],
            )

            # persistent routing state / scratch
            d_all = nc.alloc_sbuf_tensor("d_all", [128, NBCH, CPC], FP16).ap()
            wsc = nc.alloc_sbuf_tensor("wsc0", [128, NCAPS, CPC, OCH], FP16).ap()
            P = 2
            sgm = [nc.alloc_sbuf_tensor(f"sgm{p}", [128, CPC, OCH], FP16).ap() for p in range(P)]
            dds = [nc.alloc_sbuf_tensor(f"dds{p}", [128, CPC], FP16).ap() for p in range(P)]
            tmn = [nc.alloc_sbuf_tensor(f"tmn{p}", [128, CPC], FP16).ap() for p in range(P)]
            vts = [nc.alloc_sbuf_tensor(f"vt{s}", [128, NCAPS, OCH], FP16).ap() for s in range(NSL)]
            # pair-level tensors, one slot per pair, z = chunk-in-pair
            QP = 4
            Upr = [nc.alloc_sbuf_tensor(f"Upr{q}", [128, 2, NCAPS, OCH], FP16).ap() for q in range(QP)]
            ssp = [nc.alloc_sbuf_tensor(f"ssp{q}", [128, 2, NCAPS, OCH], FP16).ap() for q in range(QP)]
            pfp = [nc.alloc_sbuf_tensor(f"pfp{q}", [128, 2, NCAPS, OCH], FP16).ap() for q in range(QP)]
            s2p = [nc.alloc_sbuf_tensor(f"s2p{q}", [128, 2, NCAPS, OCH], FP32).ap() for q in range(QP)]
            sab = [nc.alloc_sbuf_tensor(f"sab{q}", [128, 2, NO], FP16).ap() for q in range(QP)]
            sqp = [nc.alloc_sbuf_tensor(f"sqp{q}", [128, 4], FP32).ap() for q in range(QP)]
            rsq = [nc.alloc_sbuf_tensor(f"rsq{q}", [128, 4], FP32).ap() for q in range(QP)]
            nt1 = [nc.alloc_sbuf_tensor(f"nt1{q}", [128, 4], FP32).ap() for q in range(QP)]
            dnp = [nc.alloc_sbuf_tensor(f"dnp{q}", [128, 4], FP32).ap() for q in range(QP)]
            rcp = [nc.alloc_sbuf_tensor(f"rcp{q}", [128, 4], FP32).ap() for q in range(QP)]
            gsq = [nc.alloc_sbuf_tensor(f"gsq{q}", [128, 4], FP32).ap() for q in range(QP)]
            fsg = [nc.alloc_sbuf_tensor(f"fsg{q}", [128, 4], FP32).ap() for q in range(QP)]
            vfp = [nc.alloc_sbuf_tensor(f"vfp{q}", [128, 2, NCAPS, OCH], FP32).ap() for q in range(QP)]

            # consts: rsqrt bithack + signed factor scales per (z, n)
            onei = nc.alloc_sbuf_tensor("onei", [128, 4], I32).ap()
            magi = nc.alloc_sbuf_tensor("magi", [128, 4], I32).ap()
            nc.vector.memset(onei[:], 1)
            nc.vector.memset(magi[:], 0x5F3759DF)

            def allreduce_pair(src, dst):
                bi = drpool.tile([128, 2 * NO], FP16, tag="bi")
                bo = drpool.tile([128, 2 * NO], FP16, tag="bo", addr_space="Shared")
                nc.sync.dma_start(bi[:], src)
                nc.gpsimd.collective_compute(
                    "AllReduce",
                    ADD,
                    replica_groups=[list(range(NCORES))],
                    ins=[bi[:].opt()],
                    outs=[bo[:].opt()],
                )
                nc.sync.dma_start(dst, bo[:])

            def tree(t, axis_len, o_axis):
                """in-place binary-tree sum over c (o_axis=False) or o."""
                lv = axis_len // 2
                while lv >= 1:
                    if o_axis:
                        a = t[:, :, :, 0:lv]
                        b = t[:, :, :, lv : 2 * lv]
                    else:
                        a = t[:, :, 0:lv, :]
                        b = t[:, :, lv : 2 * lv, :]
                    nc.vector.tensor_tensor(a, a, b, op=ADD)
                    lv //= 2

            def gemm_chunk(bk, us, with_sacc):
                u = upool.tile([128, NCAPS, CPC, OCH], FP16, tag="u")
                us[bk] = u
                if with_sacc:
                    sacc = sapool.tile([128, NO], FP32, tag="sacc", name="sacc")
                else:
                    sacc = None
                for cg in range(CPC // CG):
                    c0 = cg * CG
                    xtt = []
                    for h in range(KH):
                        t = xpool.tile([128, CG, BCH], FP16, tag=f"x{h}")
                        nc.sync.dma_start(t[:], xt[h, :, bk, c0 : c0 + CG, :])
                        xtt.append(t)
                    pg = pspool.tile([BCH, CG, NO], FP32, tag="pg")
                    for j in range(CG):
                        c = c0 + j
                        for h in range(KH):
                            lhs = xtt[h][:, j, :]
                            rhs = wsb[h][:, c * NO : (c + 1) * NO]
                            nc.tensor.matmul(
                                pg[:, j, :], lhsT=lhs, rhs=rhs,
                                start=(h == 0), stop=(h == KH - 1),
                            )
                            if with_sacc:
                                nc.tensor.matmul(
                                    sacc[:], lhsT=lhs, rhs=rhs,
                                    start=(c == 0 and h == 0),
                                    stop=(c == CPC - 1 and h == KH - 1),
                                    skip_group_check=True,
                                )
                    nc.scalar.copy(
                        u[:, :, c0 : c0 + CG, :],
                        pg[:].rearrange("b c (n o) -> b n c o", n=NCAPS),
                    )
                return u, sacc

            def utree_dve(q, z, us):
                """chunk U-partial on DVE (copy + c-tree) into sab."""
                qp = q % QP
                nc.vector.tensor_copy(wsc[:], us[2 * q + z][:])
                tree(wsc, CPC, False)
                nc.vector.tensor_copy(
                    sab[qp][:, z, :].rearrange("p (n o) -> p n o", n=NCAPS),
                    wsc[:, :, 0, :],
                )

            def sacc_out(q, z, sacc):
                nc.scalar.copy(sab[q % QP][:, z, :], sacc[:])

            def ar0(q):
                qp = q % QP
                allreduce_pair(
                    sab[qp][:].rearrange("p z f -> p (z f)"),
                    Upr[qp][:].rearrange("p z n o -> p (z n o)"),
                )

            def factor_prep(q, s_in, qscale, niter=1, fold=None):
                """vts = s_in/SC (per z); f = fold*sqrt(sq)/(1+sq) on DVE
                only (bithack rsqrt + niter Newton steps); f unsigned,
                sign applied at the d-update via op1=SUB. fold=None ->
                final exact factor into gsq (no vts)."""
                qp = q % QP
                if fold is not None:
                    for z in range(2):
                        bk = 2 * q + z
                        nc.vector.tensor_scalar(
                            vts[bk % NSL][:], s_in[:, z], 1.0 / SC, None, op0=MULT
                        )
                # sqp[:, 2z+n] = sum_o s^2 : square then grouped reduce
                s2 = s2p[qp][:].rearrange("p z n o -> p (z n) o")
                si = s_in.rearrange("p z n o -> p (z n) o")
                nc.vector.tensor_tensor(s2, si, si, op=MULT)
                nc.vector.tensor_reduce(
                    sqp[qp][:], s2, axis=mybir.AxisListType.X, op=ADD
                )
                # r ~= rsqrt(sqp): magic - (bits >> 1), Newton steps
                rb, qb = rsq[qp][:].bitcast(I32), sqp[qp][:].bitcast(I32)
                nc.vector.tensor_tensor(rb, qb, onei[:], op=SHR)
                nc.vector.tensor_tensor(rb, magi[:], rb, op=SUB)
                for _ in range(niter):
                    nc.vector.tensor_tensor(nt1[qp][:], rsq[qp][:], rsq[qp][:], op=MULT)
                    nc.vector.tensor_tensor(nt1[qp][:], nt1[qp][:], sqp[qp][:], op=MULT)
                    nc.vector.tensor_scalar(
                        nt1[qp][:], nt1[qp][:], -0.5, 1.5, op0=MULT, op1=ADD
                    )
                    nc.vector.tensor_tensor(rsq[qp][:], rsq[qp][:], nt1[qp][:], op=MULT)
                # g = sqrt(sqp); dn = (1 + qscale*sqp)/fold; f = g*recip(dn)
                fd = 1.0 if fold is None else fold
                nc.vector.tensor_tensor(gsq[qp][:], sqp[qp][:], rsq[qp][:], op=MULT)
                nc.vector.tensor_scalar(
                    dnp[qp][:], sqp[qp][:], float(qscale) / fd, 1.0 / fd,
                    op0=MULT, op1=ADD,
                )
                nc.vector.reciprocal(rcp[qp][:], dnp[qp][:])
                nc.vector.tensor_tensor(fsg[qp][:], gsq[qp][:], rcp[qp][:], op=MULT)

            def step_S0(q):
                """iteration-0: vts = U/SC; |f0| = 0.25*SC*(2*factor0)."""
                qp = q % QP
                factor_prep(q, Upr[qp][:], 0.25, fold=0.25 * SC)

            def step_A_y(q, it):
                """y-pass: t = u*vts_bcast, o-tree, d += f0*t0 + f1*t1;
                then sigma on ACT."""
                qp = q % QP
                for z in range(2):
                    bk = 2 * q + z
                    u = us[bk]
                    vb = vts[bk % NSL][:].unsqueeze(2).broadcast_to((128, NCAPS, CPC, OCH))
                    nc.vector.tensor_tensor(wsc[:], u[:], vb, op=MULT)
                    tree(wsc, OCH, True)
                    t0 = wsc[:, 0, :, 0:1]
                    t1 = wsc[:, 1, :, 0:1]
                    nc.vector.tensor_scalar(
                        tmn[z][:].unsqueeze(2), t1, fsg[qp][:, 2 * z + 1 : 2 * z + 2],
                        None, op0=MULT,
                    )
                    if it == 1:
                        nc.vector.scalar_tensor_tensor(
                            d_all[:, bk, :].unsqueeze(2), t0,
                            fsg[qp][:, 2 * z : 2 * z + 1],
                            tmn[z][:].unsqueeze(2), op0=MULT, op1=SUB,
                        )
                    else:
                        nc.vector.scalar_tensor_tensor(
                            dds[z][:].unsqueeze(2), t0,
                            fsg[qp][:, 2 * z : 2 * z + 1],
                            tmn[z][:].unsqueeze(2), op0=MULT, op1=SUB,
                        )
                        nc.vector.tensor_tensor(
                            d_all[:, bk, :], d_all[:, bk, :], dds[z][:], op=ADD
                        )
                for z in range(2):
                    bk = 2 * q + z
                    db = d_all[:, bk, :].unsqueeze(2).broadcast_to((128, CPC, OCH))
                    nc.scalar.activation(sgm[z][:], db, AF.Sigmoid)

            def step_A_s(q, it):
                """s-pass: sigma-weighted sums over c, partial extract, AR."""
                qp = q % QP
                for z in range(2):
                    sb = sgm[z][:].unsqueeze(1).broadcast_to((128, NCAPS, CPC, OCH))
                    nc.vector.tensor_tensor(wsc[:], us[2 * q + z][:], sb, op=MULT)
                    tree(wsc, CPC, False)
                    nc.scalar.copy(pfp[qp][:, z], wsc[:, :, 0, :])
                allreduce_pair(
                    pfp[qp][:].rearrange("p z n o -> p (z n o)"),
                    ssp[qp][:].rearrange("p z n o -> p (z n o)"),
                )

            def step_B(q, it):
                """post-AR: sigmoid-complement fix; factor prep (or final)."""
                qp = q % QP
                nc.vector.tensor_tensor(
                    ssp[qp][:, :, 1, :], Upr[qp][:, :, 1, :], ssp[qp][:, :, 1, :],
                    op=SUB,
                )
                if it < ITERATIONS - 1:
                    factor_prep(q, ssp[qp][:], 1.0, fold=SC)
                else:
                    factor_prep(q, ssp[qp][:], 1.0, niter=2, fold=None)
                    mb = (
                        fsg[qp][:]
                        .rearrange("p (z n) -> p z n", z=2)
                        .unsqueeze(3)
                        .broadcast_to((128, 2, NCAPS, OCH))
                    )
                    nc.vector.tensor_tensor(vfp[qp][:], ssp[qp][:], mb, op=MULT)
                    for z in range(2):
                        bk = 2 * q + z
                        nc.sync.dma_start(
                            out[bk * BCH : (bk + 1) * BCH, :, :], vfp[qp][:, z]
                        )

            # ---- schedule: emission matched to per-FIFO readiness order
            us = {}
            gemm_chunk(0, us, False); utree_dve(0, 0, us)
            gemm_chunk(1, us, False); utree_dve(0, 1, us); ar0(0)
            gemm_chunk(2, us, False)
            gemm_chunk(3, us, False)
            step_S0(0)
            step_A_y(0, 1)
            utree_dve(1, 0, us); utree_dve(1, 1, us); ar0(1)
            step_A_s(0, 1)
            _, sc4 = gemm_chunk(4, us, True); sacc_out(2, 0, sc4)
            _, sc5 = gemm_chunk(5, us, True); sacc_out(2, 1, sc5); ar0(2)
            step_S0(1); step_A_y(1, 1); step_A_s(1, 1)
            step_B(0, 1); step_A_y(0, 2)
            _, sc6 = gemm_chunk(6, us, True); sacc_out(3, 0, sc6)
            step_A_s(0, 2)
            _, sc7 = gemm_chunk(7, us, True); sacc_out(3, 1, sc7); ar0(3)
            step_B(1, 1); step_A_y(1, 2); step_A_s(1, 2)
            step_B(0, 2)
            step_S0(2); step_A_y(2, 1); step_A_s(2, 1)
            step_B(1, 2)
            step_S0(3); step_A_y(3, 1); step_A_s(3, 1)
            step_B(2, 1); step_A_y(2, 2); step_A_s(2, 2)
            step_B(3, 1); step_A_y(3, 2); step_A_s(3, 2)
            step_B(2, 2)
            step_B(3, 2)

    if split_waits:
        _split_sync_waits(nc)
    return nc


def _prep_inputs(x, W):
    x = np.ascontiguousarray(x, dtype=np.float32)
    W0 = np.ascontiguousarray(W.reshape(NCAPS, C, OCH, ICH), dtype=np.float32)
    xt_cores, wt_cores = [], []
    for k in range(NCORES):
        cs = k * CPC
        xc = x[:, cs : cs + CPC, :]  # (B, 64, 256)
        x6 = xc.reshape(NBCH, BCH, CPC, KH, 128)
        xtc = np.ascontiguousarray(x6.transpose(3, 4, 0, 2, 1)).astype(np.float16)
        xt_cores.append(xtc)
        Wc = W0[:, cs : cs + CPC]  # (2, 64, 64, 256) [n,c,o,i]
        w5 = Wc.reshape(NCAPS, CPC, OCH, KH, 128)
        wtc = np.ascontiguousarray(w5.transpose(3, 4, 1, 0, 2)).reshape(
            KH, 128, CPC, NO
        ).astype(np.float16)
        wt_cores.append(wtc)
    return xt_cores, wt_cores


_NC_CACHE = {}


def kernel(x, W):
    global LAST_EXEC_NS
    _install_profile_hook()
    if "nc" not in _NC_CACHE:
        _NC_CACHE["nc"] = build_kernel()
    nc = _NC_CACHE["nc"]
    xtc, wtc = _prep_inputs(np.asarray(x), np.asarray(W))
    in_maps = [{"xt": xtc[k], "wt": wtc[k]} for k in range(NCORES)]
    trace = bool(os.environ.get("CAPS_TRACE"))
    res = run_bass_kernel_spmd(nc, in_maps, list(range(NCORES)), trace=trace)
    LAST_EXEC_NS = res.exec_time_ns
    return res.results[0]["out"].astype(np.float32)
